# revision 4
# baseline (speedup 1.0000x reference)
"""CVKAN layer Trainium2 kernel.

Math (per reference):
    basis[b, i, k] = exp(-((x_part[b,i] - grid[k%8]) / h)^2),  part = re if k<8 else im
    out_re[b, o]   = sum_{i,k} basis[b,i,k] * coeffs_re[i,o,k] + bias_re[o]
    out_im[b, o]   = sum_{i,k} basis[b,i,k] * coeffs_im[i,o,k] + bias_im[o]
    out = out_re + 1j*out_im   (complex64)

Device strategy (data-parallel over batch, 8 cores):
  - Load x_re/x_im tiles [128b, 64i], PE-transpose the concatenated
    [128b, 128(i_re|i_im)] block -> T [128(part,i), 128b] so the
    contraction index lives on partitions.
  - For each grid point j (8 of them): one chunk of the contraction.
    basis_j = exp(-((T - g_j)/h)^2) computed on ScalarE over [128, BT]
    tiles (grid bias is a plain scalar immediate per chunk).
  - Accumulate out^T[32, b] += W_j^T @ basis_j on TensorE (weights
    stationary, 8 chunks into one PSUM accumulation group), plus one
    rank-1 matmul adding the bias.
  - Store out^T [32, 8192] fp32 per core; host interleaves to complex64
    during the gather/unshard step.
"""

import sys

import numpy as np

if "/opt/trn_rl_repo" not in sys.path:
    sys.path.append("/opt/trn_rl_repo")

B = 65536
IN = 64
OUT = 16
NB = 8
N_CORES = 8
B_CORE = B // N_CORES  # 8192
BT = 2048              # free-dim of the big working tiles
N_BIG = B_CORE // BT   # 4
H = 2.0 / (NB - 1)
GRID = [-1.0 + j * H for j in range(NB)]

# Basis mode: "square_exp" (2 ACT passes, exact) or "derf" (1 ACT pass via
# Derivative_Erf = 2/sqrt(pi) * exp(-x^2), scale folded into W host-side).
BASIS_MODE = "square_exp"

_CACHE = {}


def _build_module():
    import concourse.mybir as mybir
    import concourse.tile as tile
    from concourse import bacc
    from concourse.masks import make_identity

    f32 = mybir.dt.float32
    nc = bacc.Bacc("TRN2", target_bir_lowering=False, debug=False,
                   num_devices=N_CORES)

    x_re = nc.dram_tensor("x_re", [B_CORE, IN], f32, kind="ExternalInput")
    x_im = nc.dram_tensor("x_im", [B_CORE, IN], f32, kind="ExternalInput")
    w = nc.dram_tensor("w", [NB, 128, 2 * OUT], f32, kind="ExternalInput")
    bias32 = nc.dram_tensor("bias32", [1, 2 * OUT], f32, kind="ExternalInput")
    out_t = nc.dram_tensor("out_t", [2 * OUT, B_CORE], f32,
                           kind="ExternalOutput")

    Square = mybir.ActivationFunctionType.Square
    Exp = mybir.ActivationFunctionType.Exp
    DErf = getattr(mybir.ActivationFunctionType, "Derivative_Erf", None)

    with tile.TileContext(nc) as tc:
        with (
            tc.tile_pool(name="consts", bufs=1) as consts,
            tc.tile_pool(name="xin", bufs=2) as xpool,
            tc.tile_pool(name="tpsum", bufs=2, space="PSUM") as tpsum,
            tc.tile_pool(name="tsb", bufs=2) as tpool,
            tc.tile_pool(name="zsq", bufs=2) as zpool,
            tc.tile_pool(name="basis", bufs=3) as bpool,
            tc.tile_pool(name="opsum", bufs=1, space="PSUM") as opsum,
            tc.tile_pool(name="osb", bufs=2) as opool,
        ):
            identity = consts.tile([128, 128], f32)
            make_identity(nc, identity)
            w_sb = consts.tile([128, NB * 2 * OUT], f32)
            nc.sync.dma_start(
                out=w_sb[:].rearrange("p (j o) -> p j o", j=NB),
                in_=w.ap().rearrange("j p o -> p j o"),
            )
            bias_sb = consts.tile([1, 2 * OUT], f32)
            nc.sync.dma_start(out=bias_sb[:], in_=bias32.ap())
            ones = consts.tile([1, 512], f32)
            nc.vector.memset(ones[:], 1.0)
            # Per-chunk activation bias columns: bias_j = -grid[j]/h.
            gbias = consts.tile([128, NB], f32)
            for j in range(NB):
                nc.vector.memset(gbias[:, j:j + 1], -GRID[j] / H)

            for g in range(N_BIG):
                xcat = xpool.tile([128, BT], f32)
                xv = xcat[:].rearrange("p (nb c) -> p nb c", c=128)
                nc.sync.dma_start(
                    out=xv[:, :, 0:IN],
                    in_=x_re.ap()[g * BT:(g + 1) * BT, :]
                        .rearrange("(nb p) i -> p nb i", p=128),
                )
                nc.sync.dma_start(
                    out=xv[:, :, IN:128],
                    in_=x_im.ap()[g * BT:(g + 1) * BT, :]
                        .rearrange("(nb p) i -> p nb i", p=128),
                )

                T = tpool.tile([128, BT], f32)
                for q in range(BT // 512):
                    tp = tpsum.tile([128, 512], f32)
                    for r in range(4):
                        nb = q * 4 + r
                        nc.tensor.transpose(
                            tp[:, r * 128:(r + 1) * 128],
                            xcat[:, nb * 128:(nb + 1) * 128],
                            identity,
                        )
                    nc.vector.tensor_copy(T[:, q * 512:(q + 1) * 512], tp[:])

                out_ps = opsum.tile([2 * OUT, BT], f32)
                for j in range(NB):
                    basis = bpool.tile([128, BT], f32)
                    if BASIS_MODE == "square_exp":
                        zsq = zpool.tile([128, BT], f32)
                        nc.scalar.activation(zsq[:], T[:], Square,
                                             bias=gbias[:, j:j + 1],
                                             scale=1.0 / H)
                        nc.scalar.activation(basis[:], zsq[:], Exp, scale=-1.0)
                    elif BASIS_MODE == "derf":
                        nc.scalar.activation(basis[:], T[:], DErf,
                                             bias=gbias[:, j:j + 1],
                                             scale=1.0 / H)
                    else:
                        raise ValueError(BASIS_MODE)
                    for s in range(BT // 512):
                        nc.tensor.matmul(
                            out_ps[:, s * 512:(s + 1) * 512],
                            w_sb[:, j * 2 * OUT:(j + 1) * 2 * OUT],
                            basis[:, s * 512:(s + 1) * 512],
                            start=(j == 0),
                            stop=False,
                        )
                for s in range(BT // 512):
                    nc.tensor.matmul(
                        out_ps[:, s * 512:(s + 1) * 512],
                        bias_sb[:],
                        ones[:],
                        start=False,
                        stop=True,
                    )
                out_sb = opool.tile([2 * OUT, BT], f32)
                nc.vector.tensor_copy(out_sb[:], out_ps[:])
                nc.sync.dma_start(
                    out=out_t.ap()[:, g * BT:(g + 1) * BT], in_=out_sb[:]
                )

    nc.compile()
    return nc


def _get_module():
    if "nc" not in _CACHE:
        _CACHE["nc"] = _build_module()
    return _CACHE["nc"]


def _build_w(coeffs_re, coeffs_im):
    w = np.empty((NB, 128, 2 * OUT), dtype=np.float32)
    w[:, :IN, :OUT] = np.transpose(coeffs_re[:, :, :NB], (2, 0, 1))
    w[:, :IN, OUT:] = np.transpose(coeffs_im[:, :, :NB], (2, 0, 1))
    w[:, IN:, :OUT] = np.transpose(coeffs_re[:, :, NB:], (2, 0, 1))
    w[:, IN:, OUT:] = np.transpose(coeffs_im[:, :, NB:], (2, 0, 1))
    if BASIS_MODE == "derf":
        w *= np.float32(np.sqrt(np.pi) / 2.0)
    return w


def kernel(x_re, x_im, coeffs_re, coeffs_im, bias_re, bias_im):
    from concourse.bass_utils import run_bass_kernel_spmd

    nc = _get_module()
    w = _build_w(np.asarray(coeffs_re), np.asarray(coeffs_im))
    bias32 = np.concatenate(
        [np.asarray(bias_re), np.asarray(bias_im)]
    ).astype(np.float32).reshape(1, 2 * OUT)

    x_re = np.ascontiguousarray(x_re, dtype=np.float32)
    x_im = np.ascontiguousarray(x_im, dtype=np.float32)
    in_maps = [
        {
            "x_re": x_re[c * B_CORE:(c + 1) * B_CORE],
            "x_im": x_im[c * B_CORE:(c + 1) * B_CORE],
            "w": w,
            "bias32": bias32,
        }
        for c in range(N_CORES)
    ]
    res = run_bass_kernel_spmd(nc, in_maps, core_ids=list(range(N_CORES)))
    out = np.empty((B, OUT), dtype=np.complex64)
    for c in range(N_CORES):
        ot = res.results[c]["out_t"]  # [32, B_CORE] fp32
        out[c * B_CORE:(c + 1) * B_CORE] = (ot[:OUT].T + 1j * ot[OUT:].T)
    return out


# revision 5
# speedup vs baseline: 1.0331x; 1.0331x over previous
"""CVKAN layer Trainium2 kernel.

Math (per reference):
    basis[b, i, k] = exp(-((x_part[b,i] - grid[k%8]) / h)^2),  part = re if k<8 else im
    out_re[b, o]   = sum_{i,k} basis[b,i,k] * coeffs_re[i,o,k] + bias_re[o]
    out_im[b, o]   = sum_{i,k} basis[b,i,k] * coeffs_im[i,o,k] + bias_im[o]
    out = out_re + 1j*out_im   (complex64)

Device strategy (data-parallel over batch, 8 cores):
  - Load x_re/x_im tiles [128b, 64i], PE-transpose the concatenated
    [128b, 128(i_re|i_im)] block -> T [128(part,i), 128b] so the
    contraction index lives on partitions.
  - For each grid point j (8 of them): one chunk of the contraction.
    basis_j = exp(-((T - g_j)/h)^2) computed on ScalarE over [128, BT]
    tiles (grid bias is a plain scalar immediate per chunk).
  - Accumulate out^T[32, b] += W_j^T @ basis_j on TensorE (weights
    stationary, 8 chunks into one PSUM accumulation group), plus one
    rank-1 matmul adding the bias.
  - Store out^T [32, 8192] fp32 per core; host interleaves to complex64
    during the gather/unshard step.
"""

import sys

import numpy as np

if "/opt/trn_rl_repo" not in sys.path:
    sys.path.append("/opt/trn_rl_repo")

B = 65536
IN = 64
OUT = 16
NB = 8
N_CORES = 8
B_CORE = B // N_CORES  # 8192
BT = 2048              # free-dim of the big working tiles
N_BIG = B_CORE // BT   # 4
H = 2.0 / (NB - 1)
GRID = [-1.0 + j * H for j in range(NB)]

# Basis mode: "square_exp" (2 ACT passes, exact) or "derf" (1 ACT pass via
# Derivative_Erf = 2/sqrt(pi) * exp(-x^2), scale folded into W host-side).
BASIS_MODE = "derf"

_CACHE = {}


def _build_module():
    import concourse.mybir as mybir
    import concourse.tile as tile
    from concourse import bacc
    from concourse.masks import make_identity

    f32 = mybir.dt.float32
    nc = bacc.Bacc("TRN2", target_bir_lowering=False, debug=False,
                   num_devices=N_CORES)

    x_re = nc.dram_tensor("x_re", [B_CORE, IN], f32, kind="ExternalInput")
    x_im = nc.dram_tensor("x_im", [B_CORE, IN], f32, kind="ExternalInput")
    w = nc.dram_tensor("w", [NB, 128, 2 * OUT], f32, kind="ExternalInput")
    bias32 = nc.dram_tensor("bias32", [1, 2 * OUT], f32, kind="ExternalInput")
    out_t = nc.dram_tensor("out_t", [2 * OUT, B_CORE], f32,
                           kind="ExternalOutput")

    Square = mybir.ActivationFunctionType.Square
    Exp = mybir.ActivationFunctionType.Exp
    DErf = getattr(mybir.ActivationFunctionType, "Derivative_Erf", None)

    with tile.TileContext(nc) as tc:
        with (
            tc.tile_pool(name="consts", bufs=1) as consts,
            tc.tile_pool(name="xin", bufs=2) as xpool,
            tc.tile_pool(name="tpsum", bufs=2, space="PSUM") as tpsum,
            tc.tile_pool(name="tsb", bufs=2) as tpool,
            tc.tile_pool(name="zsq", bufs=2) as zpool,
            tc.tile_pool(name="basis", bufs=3) as bpool,
            tc.tile_pool(name="opsum", bufs=1, space="PSUM") as opsum,
            tc.tile_pool(name="osb", bufs=2) as opool,
        ):
            identity = consts.tile([128, 128], f32)
            make_identity(nc, identity)
            w_sb = consts.tile([128, NB * 2 * OUT], f32)
            nc.sync.dma_start(
                out=w_sb[:].rearrange("p (j o) -> p j o", j=NB),
                in_=w.ap().rearrange("j p o -> p j o"),
            )
            bias_sb = consts.tile([1, 2 * OUT], f32)
            nc.sync.dma_start(out=bias_sb[:], in_=bias32.ap())
            ones = consts.tile([1, 512], f32)
            nc.vector.memset(ones[:], 1.0)
            # Per-chunk activation bias columns: bias_j = -grid[j]/h.
            gbias = consts.tile([128, NB], f32)
            for j in range(NB):
                nc.vector.memset(gbias[:, j:j + 1], -GRID[j] / H)

            for g in range(N_BIG):
                xcat = xpool.tile([128, BT], f32)
                xv = xcat[:].rearrange("p (nb c) -> p nb c", c=128)
                nc.sync.dma_start(
                    out=xv[:, :, 0:IN],
                    in_=x_re.ap()[g * BT:(g + 1) * BT, :]
                        .rearrange("(nb p) i -> p nb i", p=128),
                )
                nc.sync.dma_start(
                    out=xv[:, :, IN:128],
                    in_=x_im.ap()[g * BT:(g + 1) * BT, :]
                        .rearrange("(nb p) i -> p nb i", p=128),
                )

                T = tpool.tile([128, BT], f32)
                for q in range(BT // 512):
                    tp = tpsum.tile([128, 512], f32)
                    for r in range(4):
                        nb = q * 4 + r
                        nc.tensor.transpose(
                            tp[:, r * 128:(r + 1) * 128],
                            xcat[:, nb * 128:(nb + 1) * 128],
                            identity,
                        )
                    nc.vector.tensor_copy(T[:, q * 512:(q + 1) * 512], tp[:])

                out_ps = opsum.tile([2 * OUT, BT], f32)
                for j in range(NB):
                    basis = bpool.tile([128, BT], f32)
                    if BASIS_MODE == "square_exp":
                        zsq = zpool.tile([128, BT], f32)
                        nc.scalar.activation(zsq[:], T[:], Square,
                                             bias=gbias[:, j:j + 1],
                                             scale=1.0 / H)
                        nc.scalar.activation(basis[:], zsq[:], Exp, scale=-1.0)
                    elif BASIS_MODE == "derf":
                        nc.scalar.activation(basis[:], T[:], DErf,
                                             bias=gbias[:, j:j + 1],
                                             scale=1.0 / H)
                    else:
                        raise ValueError(BASIS_MODE)
                    for s in range(BT // 512):
                        nc.tensor.matmul(
                            out_ps[:, s * 512:(s + 1) * 512],
                            w_sb[:, j * 2 * OUT:(j + 1) * 2 * OUT],
                            basis[:, s * 512:(s + 1) * 512],
                            start=(j == 0),
                            stop=False,
                        )
                for s in range(BT // 512):
                    nc.tensor.matmul(
                        out_ps[:, s * 512:(s + 1) * 512],
                        bias_sb[:],
                        ones[:],
                        start=False,
                        stop=True,
                    )
                out_sb = opool.tile([2 * OUT, BT], f32)
                nc.vector.tensor_copy(out_sb[:], out_ps[:])
                nc.sync.dma_start(
                    out=out_t.ap()[:, g * BT:(g + 1) * BT], in_=out_sb[:]
                )

    nc.compile()
    return nc


def _get_module():
    if "nc" not in _CACHE:
        _CACHE["nc"] = _build_module()
    return _CACHE["nc"]


def _build_w(coeffs_re, coeffs_im):
    w = np.empty((NB, 128, 2 * OUT), dtype=np.float32)
    w[:, :IN, :OUT] = np.transpose(coeffs_re[:, :, :NB], (2, 0, 1))
    w[:, :IN, OUT:] = np.transpose(coeffs_im[:, :, :NB], (2, 0, 1))
    w[:, IN:, :OUT] = np.transpose(coeffs_re[:, :, NB:], (2, 0, 1))
    w[:, IN:, OUT:] = np.transpose(coeffs_im[:, :, NB:], (2, 0, 1))
    if BASIS_MODE == "derf":
        w *= np.float32(np.sqrt(np.pi) / 2.0)
    return w


def kernel(x_re, x_im, coeffs_re, coeffs_im, bias_re, bias_im):
    from concourse.bass_utils import run_bass_kernel_spmd

    nc = _get_module()
    w = _build_w(np.asarray(coeffs_re), np.asarray(coeffs_im))
    bias32 = np.concatenate(
        [np.asarray(bias_re), np.asarray(bias_im)]
    ).astype(np.float32).reshape(1, 2 * OUT)

    x_re = np.ascontiguousarray(x_re, dtype=np.float32)
    x_im = np.ascontiguousarray(x_im, dtype=np.float32)
    in_maps = [
        {
            "x_re": x_re[c * B_CORE:(c + 1) * B_CORE],
            "x_im": x_im[c * B_CORE:(c + 1) * B_CORE],
            "w": w,
            "bias32": bias32,
        }
        for c in range(N_CORES)
    ]
    res = run_bass_kernel_spmd(nc, in_maps, core_ids=list(range(N_CORES)))
    out = np.empty((B, OUT), dtype=np.complex64)
    for c in range(N_CORES):
        ot = res.results[c]["out_t"]  # [32, B_CORE] fp32
        out[c * B_CORE:(c + 1) * B_CORE] = (ot[:OUT].T + 1j * ot[OUT:].T)
    return out


# revision 8
# speedup vs baseline: 2.1199x; 2.0519x over previous
"""CVKAN layer Trainium2 kernel.

Math (per reference):
    basis[b, i, k] = exp(-((x_part[b,i] - grid[k%8]) / h)^2),  part = re if k<8 else im
    out_re[b, o]   = sum_{i,k} basis[b,i,k] * coeffs_re[i,o,k] + bias_re[o]
    out_im[b, o]   = sum_{i,k} basis[b,i,k] * coeffs_im[i,o,k] + bias_im[o]
    out = out_re + 1j*out_im   (complex64)

Device strategy (data-parallel over batch, 8 cores):
  - Load x_re/x_im tiles [128b, 64i], PE-transpose the concatenated
    [128b, 128(i_re|i_im)] block -> T [128(part,i), 128b] so the
    contraction index lives on partitions.
  - For each grid point j (8 of them): one chunk of the contraction.
    basis_j = exp(-((T - g_j)/h)^2) computed on ScalarE over [128, BT]
    tiles (grid bias is a plain scalar immediate per chunk).
  - Accumulate out^T[32, b] += W_j^T @ basis_j on TensorE (weights
    stationary, 8 chunks into one PSUM accumulation group), plus one
    rank-1 matmul adding the bias.
  - Store out^T [32, 8192] fp32 per core; host interleaves to complex64
    during the gather/unshard step.
"""

import sys

import numpy as np

if "/opt/trn_rl_repo" not in sys.path:
    sys.path.append("/opt/trn_rl_repo")

B = 65536
IN = 64
OUT = 16
NB = 8
N_CORES = 8
B_CORE = B // N_CORES  # 8192
BT = 2048              # free-dim of the big working tiles
N_BIG = B_CORE // BT   # 4
H = 2.0 / (NB - 1)
GRID = [-1.0 + j * H for j in range(NB)]

# Basis mode: "square_exp" (2 ACT passes, exact) or "derf" (1 ACT pass via
# Derivative_Erf = 2/sqrt(pi) * exp(-x^2), scale folded into W host-side).
BASIS_MODE = "derf"

_CACHE = {}


def _build_module():
    import concourse.mybir as mybir
    import concourse.tile as tile
    from concourse import bacc
    from concourse.masks import make_identity

    f32 = mybir.dt.float32
    f32r = mybir.dt.float32r
    nc = bacc.Bacc("TRN2", target_bir_lowering=False, debug=False,
                   num_devices=N_CORES)

    x_re = nc.dram_tensor("x_re", [B_CORE, IN], f32, kind="ExternalInput")
    x_im = nc.dram_tensor("x_im", [B_CORE, IN], f32, kind="ExternalInput")
    w = nc.dram_tensor("w", [NB, 128, 2 * OUT], f32r, kind="ExternalInput")
    bias32 = nc.dram_tensor("bias32", [1, 2 * OUT], f32, kind="ExternalInput")
    out_t = nc.dram_tensor("out_t", [2 * OUT, B_CORE], f32,
                           kind="ExternalOutput")

    Square = mybir.ActivationFunctionType.Square
    Exp = mybir.ActivationFunctionType.Exp
    DErf = getattr(mybir.ActivationFunctionType, "Derivative_Erf", None)

    with tile.TileContext(nc) as tc:
        with (
            tc.tile_pool(name="consts", bufs=1) as consts,
            tc.tile_pool(name="xin", bufs=2) as xpool,
            tc.tile_pool(name="tpsum", bufs=2, space="PSUM") as tpsum,
            tc.tile_pool(name="tsb", bufs=2) as tpool,
            tc.tile_pool(name="zsq", bufs=2) as zpool,
            tc.tile_pool(name="basis", bufs=3) as bpool,
            tc.tile_pool(name="opsum", bufs=1, space="PSUM") as opsum,
            tc.tile_pool(name="osb", bufs=2) as opool,
        ):
            identity = consts.tile([128, 128], f32)
            make_identity(nc, identity)
            w_sb = consts.tile([128, NB * 2 * OUT], f32r)
            nc.sync.dma_start(
                out=w_sb[:].rearrange("p (j o) -> p j o", j=NB),
                in_=w.ap().rearrange("j p o -> p j o"),
            )
            bias_sb = consts.tile([2 * OUT, 1], f32)
            nc.sync.dma_start(out=bias_sb[:], in_=bias32.ap().rearrange("a o -> o a"))
            # Per-chunk activation bias columns: bias_j = -grid[j]/h.
            gbias = consts.tile([128, NB], f32)
            for j in range(NB):
                nc.vector.memset(gbias[:, j:j + 1], -GRID[j] / H)

            for g in range(N_BIG):
                xcat = xpool.tile([128, BT], f32)
                xv = xcat[:].rearrange("p (nb c) -> p nb c", c=128)
                nc.sync.dma_start(
                    out=xv[:, :, 0:IN],
                    in_=x_re.ap()[g * BT:(g + 1) * BT, :]
                        .rearrange("(nb p) i -> p nb i", p=128),
                )
                nc.sync.dma_start(
                    out=xv[:, :, IN:128],
                    in_=x_im.ap()[g * BT:(g + 1) * BT, :]
                        .rearrange("(nb p) i -> p nb i", p=128),
                )

                T = tpool.tile([128, BT], f32)
                for q in range(BT // 512):
                    tp = tpsum.tile([128, 512], f32)
                    for r in range(4):
                        nb = q * 4 + r
                        nc.tensor.transpose(
                            tp[:, r * 128:(r + 1) * 128],
                            xcat[:, nb * 128:(nb + 1) * 128],
                            identity,
                        )
                    nc.vector.tensor_copy(T[:, q * 512:(q + 1) * 512], tp[:])

                out_ps = opsum.tile([2 * OUT, BT], f32)
                for j in range(NB):
                    basis = bpool.tile([128, BT], f32r)
                    if BASIS_MODE == "square_exp":
                        zsq = zpool.tile([128, BT], f32)
                        nc.scalar.activation(zsq[:], T[:], Square,
                                             bias=gbias[:, j:j + 1],
                                             scale=1.0 / H)
                        nc.scalar.activation(basis[:], zsq[:], Exp, scale=-1.0)
                    elif BASIS_MODE == "derf":
                        nc.scalar.activation(basis[:], T[:], DErf,
                                             bias=gbias[:, j:j + 1],
                                             scale=1.0 / H)
                    else:
                        raise ValueError(BASIS_MODE)
                    for s in range(BT // 512):
                        nc.tensor.matmul(
                            out_ps[:, s * 512:(s + 1) * 512],
                            w_sb[:, j * 2 * OUT:(j + 1) * 2 * OUT],
                            basis[:, s * 512:(s + 1) * 512],
                            start=(j == 0),
                            stop=(j == NB - 1),
                        )
                out_sb = opool.tile([2 * OUT, BT], f32)
                nc.vector.tensor_scalar_add(out_sb[:], out_ps[:], bias_sb[:])
                nc.sync.dma_start(
                    out=out_t.ap()[:, g * BT:(g + 1) * BT], in_=out_sb[:]
                )

    nc.compile()
    return nc


def _get_module():
    if "nc" not in _CACHE:
        _CACHE["nc"] = _build_module()
    return _CACHE["nc"]


def _build_w(coeffs_re, coeffs_im):
    w = np.empty((NB, 128, 2 * OUT), dtype=np.float32)
    w[:, :IN, :OUT] = np.transpose(coeffs_re[:, :, :NB], (2, 0, 1))
    w[:, :IN, OUT:] = np.transpose(coeffs_im[:, :, :NB], (2, 0, 1))
    w[:, IN:, :OUT] = np.transpose(coeffs_re[:, :, NB:], (2, 0, 1))
    w[:, IN:, OUT:] = np.transpose(coeffs_im[:, :, NB:], (2, 0, 1))
    if BASIS_MODE == "derf":
        w *= np.float32(np.sqrt(np.pi) / 2.0)
    return w


def kernel(x_re, x_im, coeffs_re, coeffs_im, bias_re, bias_im):
    from concourse.bass_utils import run_bass_kernel_spmd

    nc = _get_module()
    w = _build_w(np.asarray(coeffs_re), np.asarray(coeffs_im))
    bias32 = np.concatenate(
        [np.asarray(bias_re), np.asarray(bias_im)]
    ).astype(np.float32).reshape(1, 2 * OUT)

    x_re = np.ascontiguousarray(x_re, dtype=np.float32)
    x_im = np.ascontiguousarray(x_im, dtype=np.float32)
    in_maps = [
        {
            "x_re": x_re[c * B_CORE:(c + 1) * B_CORE],
            "x_im": x_im[c * B_CORE:(c + 1) * B_CORE],
            "w": w,
            "bias32": bias32,
        }
        for c in range(N_CORES)
    ]
    res = run_bass_kernel_spmd(nc, in_maps, core_ids=list(range(N_CORES)))
    out = np.empty((B, OUT), dtype=np.complex64)
    for c in range(N_CORES):
        ot = res.results[c]["out_t"]  # [32, B_CORE] fp32
        out[c * B_CORE:(c + 1) * B_CORE] = (ot[:OUT].T + 1j * ot[OUT:].T)
    return out


# revision 9
# speedup vs baseline: 2.2024x; 1.0389x over previous
"""CVKAN layer Trainium2 kernel.

Math (per reference):
    basis[b, i, k] = exp(-((x_part[b,i] - grid[k%8]) / h)^2),  part = re if k<8 else im
    out_re[b, o]   = sum_{i,k} basis[b,i,k] * coeffs_re[i,o,k] + bias_re[o]
    out_im[b, o]   = sum_{i,k} basis[b,i,k] * coeffs_im[i,o,k] + bias_im[o]
    out = out_re + 1j*out_im   (complex64)

Device strategy (data-parallel over batch, 8 cores):
  - Load x_re/x_im tiles [128b, 64i], PE-transpose the concatenated
    [128b, 128(i_re|i_im)] block -> T [128(part,i), 128b] so the
    contraction index lives on partitions.
  - For each grid point j (8 of them): one chunk of the contraction.
    basis_j = exp(-((T - g_j)/h)^2) computed on ScalarE over [128, BT]
    tiles (grid bias is a plain scalar immediate per chunk).
  - Accumulate out^T[32, b] += W_j^T @ basis_j on TensorE (weights
    stationary, 8 chunks into one PSUM accumulation group), plus one
    rank-1 matmul adding the bias.
  - Store out^T [32, 8192] fp32 per core; host interleaves to complex64
    during the gather/unshard step.
"""

import sys

import numpy as np

if "/opt/trn_rl_repo" not in sys.path:
    sys.path.append("/opt/trn_rl_repo")

B = 65536
IN = 64
OUT = 16
NB = 8
N_CORES = 8
B_CORE = B // N_CORES  # 8192
BT = 2048              # free-dim of the big working tiles
N_BIG = B_CORE // BT   # 4
H = 2.0 / (NB - 1)
GRID = [-1.0 + j * H for j in range(NB)]

# Basis mode: "square_exp" (2 ACT passes, exact) or "derf" (1 ACT pass via
# Derivative_Erf = 2/sqrt(pi) * exp(-x^2), scale folded into W host-side).
BASIS_MODE = "derf"

_CACHE = {}


def _build_module():
    import concourse.mybir as mybir
    import concourse.tile as tile
    from concourse import bacc
    from concourse.masks import make_identity

    f32 = mybir.dt.float32
    f32r = mybir.dt.float32r
    nc = bacc.Bacc("TRN2", target_bir_lowering=False, debug=False,
                   num_devices=N_CORES)

    x_re = nc.dram_tensor("x_re", [B_CORE, IN], f32, kind="ExternalInput")
    x_im = nc.dram_tensor("x_im", [B_CORE, IN], f32, kind="ExternalInput")
    w = nc.dram_tensor("w", [NB, 128, 2 * OUT], f32r, kind="ExternalInput")
    bias32 = nc.dram_tensor("bias32", [1, 2 * OUT], f32, kind="ExternalInput")
    out_t = nc.dram_tensor("out_t", [2 * OUT, B_CORE], f32,
                           kind="ExternalOutput")

    Square = mybir.ActivationFunctionType.Square
    Exp = mybir.ActivationFunctionType.Exp
    DErf = getattr(mybir.ActivationFunctionType, "Derivative_Erf", None)

    with tile.TileContext(nc) as tc:
        with (
            tc.tile_pool(name="consts", bufs=1) as consts,
            tc.tile_pool(name="xin", bufs=3) as xpool,
            tc.tile_pool(name="tpsum", bufs=2, space="PSUM") as tpsum,
            tc.tile_pool(name="tsb", bufs=2) as tpool,
            tc.tile_pool(name="zsq", bufs=2) as zpool,
            tc.tile_pool(name="basis", bufs=3) as bpool,
            tc.tile_pool(name="opsum", bufs=1, space="PSUM") as opsum,
            tc.tile_pool(name="osb", bufs=2) as opool,
        ):
            identity = consts.tile([128, 128], f32)
            make_identity(nc, identity)
            w_sb = consts.tile([128, NB * 2 * OUT], f32r)
            nc.sync.dma_start(
                out=w_sb[:].rearrange("p (j o) -> p j o", j=NB),
                in_=w.ap().rearrange("j p o -> p j o"),
            )
            bias_sb = consts.tile([2 * OUT, 1], f32)
            nc.sync.dma_start(out=bias_sb[:], in_=bias32.ap().rearrange("a o -> o a"))
            # Per-chunk activation bias columns: bias_j = -grid[j]/h.
            gbias = consts.tile([128, NB], f32)
            for j in range(NB):
                nc.vector.memset(gbias[:, j:j + 1], -GRID[j] / H)

            for g in range(N_BIG):
                T = tpool.tile([128, BT], f32)
                for q in range(BT // 512):
                    # Load 4 b-blocks (512 batch rows) of x_re|x_im columns.
                    xcat = xpool.tile([128, 512], f32)
                    xv = xcat[:].rearrange("p (nb c) -> p nb c", c=128)
                    b0 = g * BT + q * 512
                    nc.sync.dma_start(
                        out=xv[:, :, 0:IN],
                        in_=x_re.ap()[b0:b0 + 512, :]
                            .rearrange("(nb p) i -> p nb i", p=128),
                    )
                    nc.sync.dma_start(
                        out=xv[:, :, IN:128],
                        in_=x_im.ap()[b0:b0 + 512, :]
                            .rearrange("(nb p) i -> p nb i", p=128),
                    )
                    tp = tpsum.tile([128, 512], f32)
                    for r in range(4):
                        nc.tensor.transpose(
                            tp[:, r * 128:(r + 1) * 128],
                            xcat[:, r * 128:(r + 1) * 128],
                            identity,
                        )
                    nc.vector.tensor_copy(T[:, q * 512:(q + 1) * 512], tp[:])

                out_ps = opsum.tile([2 * OUT, BT], f32)
                for j in range(NB):
                    basis = bpool.tile([128, BT], f32r)
                    if BASIS_MODE == "square_exp":
                        zsq = zpool.tile([128, BT], f32)
                        nc.scalar.activation(zsq[:], T[:], Square,
                                             bias=gbias[:, j:j + 1],
                                             scale=1.0 / H)
                        nc.scalar.activation(basis[:], zsq[:], Exp, scale=-1.0)
                    elif BASIS_MODE == "derf":
                        nc.scalar.activation(basis[:], T[:], DErf,
                                             bias=gbias[:, j:j + 1],
                                             scale=1.0 / H)
                    else:
                        raise ValueError(BASIS_MODE)
                    for s in range(BT // 512):
                        nc.tensor.matmul(
                            out_ps[:, s * 512:(s + 1) * 512],
                            w_sb[:, j * 2 * OUT:(j + 1) * 2 * OUT],
                            basis[:, s * 512:(s + 1) * 512],
                            start=(j == 0),
                            stop=(j == NB - 1),
                        )
                out_sb = opool.tile([2 * OUT, BT], f32)
                for u in range(2):
                    sl = slice(u * BT // 2, (u + 1) * BT // 2)
                    nc.vector.tensor_scalar_add(out_sb[:, sl], out_ps[:, sl],
                                                bias_sb[:])
                    nc.sync.dma_start(
                        out=out_t.ap()[:, g * BT + u * BT // 2:
                                       g * BT + (u + 1) * BT // 2],
                        in_=out_sb[:, sl],
                    )

    nc.compile()
    return nc


def _get_module():
    if "nc" not in _CACHE:
        _CACHE["nc"] = _build_module()
    return _CACHE["nc"]


def _build_w(coeffs_re, coeffs_im):
    w = np.empty((NB, 128, 2 * OUT), dtype=np.float32)
    w[:, :IN, :OUT] = np.transpose(coeffs_re[:, :, :NB], (2, 0, 1))
    w[:, :IN, OUT:] = np.transpose(coeffs_im[:, :, :NB], (2, 0, 1))
    w[:, IN:, :OUT] = np.transpose(coeffs_re[:, :, NB:], (2, 0, 1))
    w[:, IN:, OUT:] = np.transpose(coeffs_im[:, :, NB:], (2, 0, 1))
    if BASIS_MODE == "derf":
        w *= np.float32(np.sqrt(np.pi) / 2.0)
    return w


def kernel(x_re, x_im, coeffs_re, coeffs_im, bias_re, bias_im):
    from concourse.bass_utils import run_bass_kernel_spmd

    nc = _get_module()
    w = _build_w(np.asarray(coeffs_re), np.asarray(coeffs_im))
    bias32 = np.concatenate(
        [np.asarray(bias_re), np.asarray(bias_im)]
    ).astype(np.float32).reshape(1, 2 * OUT)

    x_re = np.ascontiguousarray(x_re, dtype=np.float32)
    x_im = np.ascontiguousarray(x_im, dtype=np.float32)
    in_maps = [
        {
            "x_re": x_re[c * B_CORE:(c + 1) * B_CORE],
            "x_im": x_im[c * B_CORE:(c + 1) * B_CORE],
            "w": w,
            "bias32": bias32,
        }
        for c in range(N_CORES)
    ]
    res = run_bass_kernel_spmd(nc, in_maps, core_ids=list(range(N_CORES)))
    out = np.empty((B, OUT), dtype=np.complex64)
    for c in range(N_CORES):
        ot = res.results[c]["out_t"]  # [32, B_CORE] fp32
        out[c * B_CORE:(c + 1) * B_CORE] = (ot[:OUT].T + 1j * ot[OUT:].T)
    return out


# revision 11
# speedup vs baseline: 2.2631x; 1.0276x over previous
"""CVKAN layer Trainium2 kernel.

Math (per reference):
    basis[b, i, k] = exp(-((x_part[b,i] - grid[k%8]) / h)^2),  part = re if k<8 else im
    out_re[b, o]   = sum_{i,k} basis[b,i,k] * coeffs_re[i,o,k] + bias_re[o]
    out_im[b, o]   = sum_{i,k} basis[b,i,k] * coeffs_im[i,o,k] + bias_im[o]
    out = out_re + 1j*out_im   (complex64)

Device strategy (data-parallel over batch, 8 cores):
  - Load x_re/x_im tiles [128b, 64i], PE-transpose the concatenated
    [128b, 128(i_re|i_im)] block -> T [128(part,i), 128b] so the
    contraction index lives on partitions.
  - For each grid point j (8 of them): one chunk of the contraction.
    basis_j = exp(-((T - g_j)/h)^2) computed on ScalarE over [128, BT]
    tiles (grid bias is a plain scalar immediate per chunk).
  - Accumulate out^T[32, b] += W_j^T @ basis_j on TensorE (weights
    stationary, 8 chunks into one PSUM accumulation group), plus one
    rank-1 matmul adding the bias.
  - Store out^T [32, 8192] fp32 per core; host interleaves to complex64
    during the gather/unshard step.
"""

import sys

import numpy as np

if "/opt/trn_rl_repo" not in sys.path:
    sys.path.append("/opt/trn_rl_repo")

B = 65536
IN = 64
OUT = 16
NB = 8
N_CORES = 8
B_CORE = B // N_CORES  # 8192
BT = 2048              # free-dim of the big working tiles
N_BIG = B_CORE // BT   # 4
# Graduated tile sizes: small first tile starts ScalarE sooner; small last
# tile shortens the matmul/copy/store tail after the final activation.
TILE_SIZES = [1024, 2048, 2048, 2048, 1024]
assert sum(TILE_SIZES) == B_CORE
H = 2.0 / (NB - 1)
GRID = [-1.0 + j * H for j in range(NB)]

# Basis mode: "square_exp" (2 ACT passes, exact) or "derf" (1 ACT pass via
# Derivative_Erf = 2/sqrt(pi) * exp(-x^2), scale folded into W host-side).
BASIS_MODE = "derf"

_CACHE = {}


def _build_module():
    import concourse.mybir as mybir
    import concourse.tile as tile
    from concourse import bacc
    from concourse.masks import make_identity

    f32 = mybir.dt.float32
    f32r = mybir.dt.float32r
    nc = bacc.Bacc("TRN2", target_bir_lowering=False, debug=False,
                   num_devices=N_CORES)

    x_re = nc.dram_tensor("x_re", [B_CORE, IN], f32, kind="ExternalInput")
    x_im = nc.dram_tensor("x_im", [B_CORE, IN], f32, kind="ExternalInput")
    w = nc.dram_tensor("w", [NB, 128, 2 * OUT], f32r, kind="ExternalInput")
    bias32 = nc.dram_tensor("bias32", [1, 2 * OUT], f32, kind="ExternalInput")
    out_t = nc.dram_tensor("out_t", [2 * OUT, B_CORE], f32,
                           kind="ExternalOutput")

    Square = mybir.ActivationFunctionType.Square
    Exp = mybir.ActivationFunctionType.Exp
    DErf = getattr(mybir.ActivationFunctionType, "Derivative_Erf", None)

    with tile.TileContext(nc) as tc:
        with (
            tc.tile_pool(name="consts", bufs=1) as consts,
            tc.tile_pool(name="xin", bufs=3) as xpool,
            tc.tile_pool(name="tpsum", bufs=2, space="PSUM") as tpsum,
            tc.tile_pool(name="tsb", bufs=2) as tpool,
            tc.tile_pool(name="zsq", bufs=2) as zpool,
            tc.tile_pool(name="basis", bufs=3) as bpool,
            tc.tile_pool(name="opsum", bufs=1, space="PSUM") as opsum,
            tc.tile_pool(name="osb", bufs=2) as opool,
        ):
            identity = consts.tile([128, 128], f32)
            make_identity(nc, identity)
            w_sb = consts.tile([128, NB * 2 * OUT], f32r)
            nc.sync.dma_start(
                out=w_sb[:].rearrange("p (j o) -> p j o", j=NB),
                in_=w.ap().rearrange("j p o -> p j o"),
            )
            bias_sb = consts.tile([2 * OUT, 1], f32)
            nc.sync.dma_start(out=bias_sb[:], in_=bias32.ap().rearrange("a o -> o a"))
            # Per-chunk activation bias columns: bias_j = -grid[j]/h.
            gbias = consts.tile([128, NB], f32)
            for j in range(NB):
                nc.vector.memset(gbias[:, j:j + 1], -GRID[j] / H)

            base = 0
            for g, bt in enumerate(TILE_SIZES):
                T = tpool.tile([128, bt], f32, tag="T")
                for q in range(bt // 512):
                    # Load 4 b-blocks (512 batch rows) of x_re|x_im columns.
                    xcat = xpool.tile([128, 512], f32)
                    xv = xcat[:].rearrange("p (nb c) -> p nb c", c=128)
                    b0 = base + q * 512
                    nc.sync.dma_start(
                        out=xv[:, :, 0:IN],
                        in_=x_re.ap()[b0:b0 + 512, :]
                            .rearrange("(nb p) i -> p nb i", p=128),
                    )
                    nc.sync.dma_start(
                        out=xv[:, :, IN:128],
                        in_=x_im.ap()[b0:b0 + 512, :]
                            .rearrange("(nb p) i -> p nb i", p=128),
                    )
                    tp = tpsum.tile([128, 512], f32)
                    for r in range(4):
                        nc.tensor.transpose(
                            tp[:, r * 128:(r + 1) * 128],
                            xcat[:, r * 128:(r + 1) * 128],
                            identity,
                        )
                    nc.vector.tensor_copy(T[:, q * 512:(q + 1) * 512], tp[:])

                out_ps = opsum.tile([2 * OUT, bt], f32, tag="out_ps")
                for j in range(NB):
                    basis = bpool.tile([128, bt], f32r, tag="basis")
                    if BASIS_MODE == "square_exp":
                        zsq = zpool.tile([128, bt], f32, tag="zsq")
                        nc.scalar.activation(zsq[:], T[:], Square,
                                             bias=gbias[:, j:j + 1],
                                             scale=1.0 / H)
                        nc.scalar.activation(basis[:], zsq[:], Exp, scale=-1.0)
                    elif BASIS_MODE == "derf":
                        nc.scalar.activation(basis[:], T[:], DErf,
                                             bias=gbias[:, j:j + 1],
                                             scale=1.0 / H)
                    else:
                        raise ValueError(BASIS_MODE)
                    for s in range(bt // 512):
                        nc.tensor.matmul(
                            out_ps[:, s * 512:(s + 1) * 512],
                            w_sb[:, j * 2 * OUT:(j + 1) * 2 * OUT],
                            basis[:, s * 512:(s + 1) * 512],
                            start=(j == 0),
                            stop=(j == NB - 1),
                        )
                out_sb = opool.tile([2 * OUT, bt], f32, tag="out_sb")
                for u in range(bt // 1024):
                    sl = slice(u * 1024, (u + 1) * 1024)
                    nc.vector.tensor_scalar_add(out_sb[:, sl], out_ps[:, sl],
                                                bias_sb[:])
                    nc.sync.dma_start(
                        out=out_t.ap()[:, base + u * 1024:base + (u + 1) * 1024],
                        in_=out_sb[:, sl],
                    )
                base += bt

    nc.compile()
    return nc


def _get_module():
    if "nc" not in _CACHE:
        _CACHE["nc"] = _build_module()
    return _CACHE["nc"]


def _build_w(coeffs_re, coeffs_im):
    w = np.empty((NB, 128, 2 * OUT), dtype=np.float32)
    w[:, :IN, :OUT] = np.transpose(coeffs_re[:, :, :NB], (2, 0, 1))
    w[:, :IN, OUT:] = np.transpose(coeffs_im[:, :, :NB], (2, 0, 1))
    w[:, IN:, :OUT] = np.transpose(coeffs_re[:, :, NB:], (2, 0, 1))
    w[:, IN:, OUT:] = np.transpose(coeffs_im[:, :, NB:], (2, 0, 1))
    if BASIS_MODE == "derf":
        w *= np.float32(np.sqrt(np.pi) / 2.0)
    return w


def kernel(x_re, x_im, coeffs_re, coeffs_im, bias_re, bias_im):
    from concourse.bass_utils import run_bass_kernel_spmd

    nc = _get_module()
    w = _build_w(np.asarray(coeffs_re), np.asarray(coeffs_im))
    bias32 = np.concatenate(
        [np.asarray(bias_re), np.asarray(bias_im)]
    ).astype(np.float32).reshape(1, 2 * OUT)

    x_re = np.ascontiguousarray(x_re, dtype=np.float32)
    x_im = np.ascontiguousarray(x_im, dtype=np.float32)
    in_maps = [
        {
            "x_re": x_re[c * B_CORE:(c + 1) * B_CORE],
            "x_im": x_im[c * B_CORE:(c + 1) * B_CORE],
            "w": w,
            "bias32": bias32,
        }
        for c in range(N_CORES)
    ]
    res = run_bass_kernel_spmd(nc, in_maps, core_ids=list(range(N_CORES)))
    out = np.empty((B, OUT), dtype=np.complex64)
    for c in range(N_CORES):
        ot = res.results[c]["out_t"]  # [32, B_CORE] fp32
        out[c * B_CORE:(c + 1) * B_CORE] = (ot[:OUT].T + 1j * ot[OUT:].T)
    return out


# revision 12
# speedup vs baseline: 2.3035x; 1.0178x over previous
"""CVKAN layer Trainium2 kernel.

Math (per reference):
    basis[b, i, k] = exp(-((x_part[b,i] - grid[k%8]) / h)^2),  part = re if k<8 else im
    out_re[b, o]   = sum_{i,k} basis[b,i,k] * coeffs_re[i,o,k] + bias_re[o]
    out_im[b, o]   = sum_{i,k} basis[b,i,k] * coeffs_im[i,o,k] + bias_im[o]
    out = out_re + 1j*out_im   (complex64)

Device strategy (data-parallel over batch, 8 cores):
  - Load x_re/x_im tiles [128b, 64i], PE-transpose the concatenated
    [128b, 128(i_re|i_im)] block -> T [128(part,i), 128b] so the
    contraction index lives on partitions.
  - For each grid point j (8 of them): one chunk of the contraction.
    basis_j = exp(-((T - g_j)/h)^2) computed on ScalarE over [128, BT]
    tiles (grid bias is a plain scalar immediate per chunk).
  - Accumulate out^T[32, b] += W_j^T @ basis_j on TensorE (weights
    stationary, 8 chunks into one PSUM accumulation group), plus one
    rank-1 matmul adding the bias.
  - Store out^T [32, 8192] fp32 per core; host interleaves to complex64
    during the gather/unshard step.
"""

import sys

import numpy as np

if "/opt/trn_rl_repo" not in sys.path:
    sys.path.append("/opt/trn_rl_repo")

B = 65536
IN = 64
OUT = 16
NB = 8
N_CORES = 8
B_CORE = B // N_CORES  # 8192
BT = 2048              # free-dim of the big working tiles
N_BIG = B_CORE // BT   # 4
# Graduated tile sizes: small first tile starts ScalarE sooner; small last
# tile shortens the matmul/copy/store tail after the final activation.
TILE_SIZES = [1024, 2048, 2048, 2048, 1024]
assert sum(TILE_SIZES) == B_CORE
H = 2.0 / (NB - 1)
GRID = [-1.0 + j * H for j in range(NB)]

# Basis mode: "square_exp" (2 ACT passes, exact) or "derf" (1 ACT pass via
# Derivative_Erf = 2/sqrt(pi) * exp(-x^2), scale folded into W host-side).
BASIS_MODE = "derf"

_CACHE = {}


def _build_module():
    import concourse.mybir as mybir
    import concourse.tile as tile
    from concourse import bacc
    from concourse.masks import make_identity

    f32 = mybir.dt.float32
    f32r = mybir.dt.float32r
    nc = bacc.Bacc("TRN2", target_bir_lowering=False, debug=False,
                   num_devices=N_CORES)

    x_re = nc.dram_tensor("x_re", [B_CORE, IN], f32, kind="ExternalInput")
    x_im = nc.dram_tensor("x_im", [B_CORE, IN], f32, kind="ExternalInput")
    w = nc.dram_tensor("w", [NB, 128, 2 * OUT], f32r, kind="ExternalInput")
    bias32 = nc.dram_tensor("bias32", [1, 2 * OUT], f32, kind="ExternalInput")
    out_t = nc.dram_tensor("out_t", [2 * OUT, B_CORE], f32,
                           kind="ExternalOutput")

    Square = mybir.ActivationFunctionType.Square
    Exp = mybir.ActivationFunctionType.Exp
    DErf = getattr(mybir.ActivationFunctionType, "Derivative_Erf", None)

    with tile.TileContext(nc) as tc:
        with (
            tc.tile_pool(name="consts", bufs=1) as consts,
            tc.tile_pool(name="xin", bufs=4) as xpool,
            tc.tile_pool(name="tpsum", bufs=3, space="PSUM") as tpsum,
            tc.tile_pool(name="tsb", bufs=3) as tpool,
            tc.tile_pool(name="zsq", bufs=2) as zpool,
            tc.tile_pool(name="basis", bufs=4) as bpool,
            tc.tile_pool(name="opsum", bufs=1, space="PSUM") as opsum,
            tc.tile_pool(name="osb", bufs=2) as opool,
        ):
            identity = consts.tile([128, 128], f32)
            make_identity(nc, identity)
            w_sb = consts.tile([128, NB * 2 * OUT], f32r)
            nc.sync.dma_start(
                out=w_sb[:].rearrange("p (j o) -> p j o", j=NB),
                in_=w.ap().rearrange("j p o -> p j o"),
            )
            bias_sb = consts.tile([2 * OUT, 1], f32)
            nc.sync.dma_start(out=bias_sb[:], in_=bias32.ap().rearrange("a o -> o a"))
            # Per-chunk activation bias columns: bias_j = -grid[j]/h.
            gbias = consts.tile([128, NB], f32)
            for j in range(NB):
                nc.vector.memset(gbias[:, j:j + 1], -GRID[j] / H)

            base = 0
            for g, bt in enumerate(TILE_SIZES):
                T = tpool.tile([128, bt], f32, tag="T")
                for q in range(bt // 512):
                    # Load 4 b-blocks (512 batch rows) of x_re|x_im columns.
                    xcat = xpool.tile([128, 512], f32)
                    xv = xcat[:].rearrange("p (nb c) -> p nb c", c=128)
                    b0 = base + q * 512
                    nc.sync.dma_start(
                        out=xv[:, :, 0:IN],
                        in_=x_re.ap()[b0:b0 + 512, :]
                            .rearrange("(nb p) i -> p nb i", p=128),
                    )
                    nc.sync.dma_start(
                        out=xv[:, :, IN:128],
                        in_=x_im.ap()[b0:b0 + 512, :]
                            .rearrange("(nb p) i -> p nb i", p=128),
                    )
                    tp = tpsum.tile([128, 512], f32)
                    for r in range(4):
                        nc.tensor.transpose(
                            tp[:, r * 128:(r + 1) * 128],
                            xcat[:, r * 128:(r + 1) * 128],
                            identity,
                        )
                    nc.vector.tensor_copy(T[:, q * 512:(q + 1) * 512], tp[:])

                out_ps = opsum.tile([2 * OUT, bt], f32, tag="out_ps")
                for j in range(NB):
                    basis = bpool.tile([128, bt], f32r, tag="basis")
                    if BASIS_MODE == "square_exp":
                        zsq = zpool.tile([128, bt], f32, tag="zsq")
                        nc.scalar.activation(zsq[:], T[:], Square,
                                             bias=gbias[:, j:j + 1],
                                             scale=1.0 / H)
                        nc.scalar.activation(basis[:], zsq[:], Exp, scale=-1.0)
                    elif BASIS_MODE == "derf":
                        nc.scalar.activation(basis[:], T[:], DErf,
                                             bias=gbias[:, j:j + 1],
                                             scale=1.0 / H)
                    else:
                        raise ValueError(BASIS_MODE)
                    for s in range(bt // 512):
                        nc.tensor.matmul(
                            out_ps[:, s * 512:(s + 1) * 512],
                            w_sb[:, j * 2 * OUT:(j + 1) * 2 * OUT],
                            basis[:, s * 512:(s + 1) * 512],
                            start=(j == 0),
                            stop=(j == NB - 1),
                        )
                out_sb = opool.tile([2 * OUT, bt], f32, tag="out_sb")
                for u in range(bt // 1024):
                    sl = slice(u * 1024, (u + 1) * 1024)
                    nc.vector.tensor_scalar_add(out_sb[:, sl], out_ps[:, sl],
                                                bias_sb[:])
                    nc.sync.dma_start(
                        out=out_t.ap()[:, base + u * 1024:base + (u + 1) * 1024],
                        in_=out_sb[:, sl],
                    )
                base += bt

    nc.compile()
    return nc


def _get_module():
    if "nc" not in _CACHE:
        _CACHE["nc"] = _build_module()
    return _CACHE["nc"]


def _build_w(coeffs_re, coeffs_im):
    w = np.empty((NB, 128, 2 * OUT), dtype=np.float32)
    w[:, :IN, :OUT] = np.transpose(coeffs_re[:, :, :NB], (2, 0, 1))
    w[:, :IN, OUT:] = np.transpose(coeffs_im[:, :, :NB], (2, 0, 1))
    w[:, IN:, :OUT] = np.transpose(coeffs_re[:, :, NB:], (2, 0, 1))
    w[:, IN:, OUT:] = np.transpose(coeffs_im[:, :, NB:], (2, 0, 1))
    if BASIS_MODE == "derf":
        w *= np.float32(np.sqrt(np.pi) / 2.0)
    return w


def kernel(x_re, x_im, coeffs_re, coeffs_im, bias_re, bias_im):
    from concourse.bass_utils import run_bass_kernel_spmd

    nc = _get_module()
    w = _build_w(np.asarray(coeffs_re), np.asarray(coeffs_im))
    bias32 = np.concatenate(
        [np.asarray(bias_re), np.asarray(bias_im)]
    ).astype(np.float32).reshape(1, 2 * OUT)

    x_re = np.ascontiguousarray(x_re, dtype=np.float32)
    x_im = np.ascontiguousarray(x_im, dtype=np.float32)
    in_maps = [
        {
            "x_re": x_re[c * B_CORE:(c + 1) * B_CORE],
            "x_im": x_im[c * B_CORE:(c + 1) * B_CORE],
            "w": w,
            "bias32": bias32,
        }
        for c in range(N_CORES)
    ]
    res = run_bass_kernel_spmd(nc, in_maps, core_ids=list(range(N_CORES)))
    out = np.empty((B, OUT), dtype=np.complex64)
    for c in range(N_CORES):
        ot = res.results[c]["out_t"]  # [32, B_CORE] fp32
        out[c * B_CORE:(c + 1) * B_CORE] = (ot[:OUT].T + 1j * ot[OUT:].T)
    return out
